# revision 1
# baseline (speedup 1.0000x reference)
"""Trainium2 Bass kernel for nn_DeepEPMoE: top-2 MoE (B=2,S=2048,D=1024,H=4096,E=8).

Expert-parallel over 8 cores (1 expert per core):
  - host replicates x to every core (so all-to-all dispatch becomes a local
    gather), slices w1/w2 by expert, slices x tokens for the router.
  - each core: router logits (fp32 matmul) on its 512-token slice -> softmax
    -> top-2 (DVE max8) -> renormalized gates -> AllGather of (g1,g2,i1,i2).
  - build the token list for this core's expert with iota/select +
    gpsimd sparse_gather compaction; tail slots point at a zero pad row.
  - dma_gather token rows, PE-transpose, bf16 fc1 -> exact Gelu -> bf16 fc2,
    scale by gate, dma_scatter_add into a zeroed [T+1, D] partial buffer.
  - ReduceScatter(add) over the 8 cores; core c returns rows [512c:512c+512).
"""

import os
import sys

import numpy as np

for _p in ("/opt/trn_rl_repo",):
    if _p not in sys.path:
        sys.path.insert(0, _p)

import concourse.bass as bass
import concourse.mybir as mybir
import concourse.tile as tile
from concourse import bacc, library_config
from concourse.bass import ds, ts
from concourse.masks import make_identity

F32 = mybir.dt.float32
BF16 = mybir.dt.bfloat16
I16 = mybir.dt.int16
I32 = mybir.dt.int32
U32 = mybir.dt.uint32
AF = mybir.ActivationFunctionType
ALU = mybir.AluOpType

REAL = dict(T=4096, D=1024, H=4096, E=8, NCORES=8, C=2048, TT=256, act="gelu")


def build_moe(p):
    T, D, H, E = p["T"], p["D"], p["H"], p["E"]
    NCORES, C, TT = p["NCORES"], p["C"], p["TT"]
    TC = T // NCORES          # tokens per core for the router
    ND = D // 128             # D chunks (contraction tiles for fc1)
    NH = H // 128             # H chunks
    NTB = TT // 128           # 128-token blocks per t-tile
    NTT = C // TT             # t-tiles over the capacity
    DT = min(512, D)          # fc2 output tile width
    NDT = D // DT
    RB = TC // 128            # router 128-token blocks
    assert TT % 128 == 0 and C % TT == 0 and T % 16 == 0 and C % 16 == 0
    assert C // 16 <= 512 and C <= T

    nc = bacc.Bacc(
        "TRN2",
        target_bir_lowering=False,
        debug=False,
        enable_asserts=False,
        num_devices=NCORES,
    )

    # ---------------- I/O ----------------
    xf = nc.dram_tensor("xf", [T + 1, D], F32, kind="ExternalInput")  # row T = zeros
    xs = nc.dram_tensor("xs", [TC, D], F32, kind="ExternalInput")     # router slice
    rwt = nc.dram_tensor("rwt", [D, E], F32, kind="ExternalInput")    # router_w.T
    w1 = nc.dram_tensor("w1", [D, H], F32, kind="ExternalInput")      # this expert
    w2 = nc.dram_tensor("w2", [H, D], F32, kind="ExternalInput")
    cid = nc.dram_tensor("cid", [128, 1], F32, kind="ExternalInput")  # core id
    iotw = nc.dram_tensor("iotw", [16, T // 16], F32, kind="ExternalInput")
    out = nc.dram_tensor("out", [TC, D], F32, kind="ExternalOutput")

    groups = [list(range(NCORES))]

    with tile.TileContext(nc) as tc:
        with (
            tc.tile_pool(name="wpool", bufs=1) as wpool,
            tc.tile_pool(name="wtmp", bufs=2) as wtmpp,
            tc.tile_pool(name="rpool", bufs=1) as rpool,
            tc.tile_pool(name="rsm", bufs=2) as rsm,
            tc.tile_pool(name="xgp", bufs=2) as xgp,
            tc.tile_pool(name="xgtp", bufs=2) as xgtp,
            tc.tile_pool(name="htp", bufs=4) as htp,
            tc.tile_pool(name="yp", bufs=1) as yp,
            tc.tile_pool(name="psA", bufs=2, space="PSUM") as psA,
            tc.tile_pool(name="psT", bufs=2, space="PSUM") as psT,
            tc.tile_pool(name="psY", bufs=1, space="PSUM") as psY,
            tc.tile_pool(name="dram", bufs=1, space="DRAM") as dram,
        ):
            # ---------------- DRAM scratch ----------------
            partial = dram.tile([T, D], F32)
            ydram = dram.tile([C, D], F32)
            rt_loc = dram.tile([TC, 4], F32)
            rt_all = dram.tile([T, 4], F32, addr_space="Shared")
            rs_out = dram.tile([TC, D], F32)

            # ---------------- constants ----------------
            ident = rpool.tile([128, 128], F32)
            make_identity(nc, ident[:])
            # strict upper-triangular ones (lhsT for per-block prefix sums)
            utri = rpool.tile([128, 128], F32)
            nc.gpsimd.memset(utri[:], 0.0)
            nc.gpsimd.affine_select(
                out=utri[:], in_=utri[:], compare_op=ALU.is_ge, fill=1.0,
                base=0, channel_multiplier=1, pattern=[[-1, 128]],
            )
            ones = rpool.tile([128, 128], F32)
            nc.gpsimd.memset(ones[:], 1.0)

            # ---------------- weights load + bf16 convert ----------------
            WCH = min(1024, H)
            w1b = wpool.tile([128, ND, H], BF16)
            w2b = wpool.tile([128, NH, D], BF16)
            for d in range(ND):
                for hh in range(H // WCH):
                    wt = wtmpp.tile([128, WCH], F32, tag="wtmp")
                    nc.sync.dma_start(wt[:], w1[ts(d, 128), ds(hh * WCH, WCH)])
                    nc.vector.tensor_copy(w1b[:, d, ds(hh * WCH, WCH)], wt[:])
            for h in range(NH):
                for dd in range(D // min(1024, D)):
                    DCH = min(1024, D)
                    wt2 = wtmpp.tile([128, DCH], F32, tag="wtmp")
                    nc.sync.dma_start(wt2[:], w2[ts(h, 128), ds(dd * DCH, DCH)])
                    nc.vector.tensor_copy(w2b[:, h, ds(dd * DCH, DCH)], wt2[:])

            # ---------------- router on this core's slice ----------------
            rwt_sb = rpool.tile([128, ND, E], F32)
            nc.sync.dma_start(
                rwt_sb[:], rwt[:].rearrange("(nd p) e -> p nd e", p=128)
            )
            rt_sb = rpool.tile([128, RB, 4], F32)
            for f in range(RB):
                # load+transpose this 128-token block: xsT [128(d), ND, 128(t)]
                # (tag-share with the later t-tile loop tiles to save SBUF)
                xs_sb = xgp.tile([128, D], F32, tag="xg")
                nc.sync.dma_start(xs_sb[:], xs[ts(f, 128), :])
                xsT = yp.tile([128, ND, 128], F32, tag="y")
                for d in range(ND):
                    pt = psT.tile([128, 128], F32, tag="psT")
                    nc.tensor.transpose(pt[:], xs_sb[:, ts(d, 128)], ident[:])
                    nc.scalar.copy(xsT[:, d, :], pt[:])
                pl = psA.tile([128, E], F32, tag="psA")
                for d in range(ND):
                    nc.tensor.matmul(
                        pl[:], xsT[:, d, :], rwt_sb[:, d, :],
                        start=(d == 0), stop=(d == ND - 1),
                    )
                # softmax over E (free axis), renormalized top-2
                nmax = rsm.tile([128, 1], F32, tag="nmax")
                nc.vector.reduce_max(
                    nmax[:], pl[:], axis=mybir.AxisListType.X, negate=True
                )
                probs = rsm.tile([128, E], F32, tag="probs")
                zsum = rsm.tile([128, 1], F32, tag="zsum")
                nc.scalar.activation(
                    probs[:], pl[:], AF.Exp, bias=nmax[:], accum_out=zsum[:]
                )
                zinv = rsm.tile([128, 1], F32, tag="zinv")
                nc.vector.reciprocal(zinv[:], zsum[:])
                nc.vector.tensor_scalar_mul(probs[:], probs[:], zinv[:])
                vmax = rsm.tile([128, 8], F32, tag="vmax")
                vidx = rsm.tile([128, 8], U32, tag="vidx")
                nc.vector.max(vmax[:], probs[:])
                nc.vector.max_index(vidx[:], vmax[:], probs[:])
                ssum = rsm.tile([128, 1], F32, tag="ssum")
                nc.vector.tensor_tensor(
                    ssum[:], vmax[:, 0:1], vmax[:, 1:2], ALU.add
                )
                nc.vector.tensor_scalar_add(ssum[:], ssum[:], 1e-9)
                sinv = rsm.tile([128, 1], F32, tag="sinv")
                nc.vector.reciprocal(sinv[:], ssum[:])
                nc.vector.tensor_scalar(
                    rt_sb[:, f, 0:2], vmax[:, 0:2], sinv[:], None, ALU.mult
                )
                nc.vector.tensor_copy(rt_sb[:, f, 2:4], vidx[:, 0:2])
            # rt_loc[t, v] = rt_sb[t%128, t//128, v]
            nc.sync.dma_start(
                rt_loc[:].rearrange("(f q) v -> q f v", q=128), rt_sb[:]
            )
            nc.gpsimd.collective_compute(
                "AllGather", ALU.bypass, replica_groups=groups,
                ins=[rt_loc[:].opt()], outs=[rt_all[:].opt()],
            )

            # ---------------- routing masks / gates / positions ----------------
            F16 = T // 16   # [16, F16] token wrap for sparse_gather
            CF = C // 16
            NF = T // 128   # token-major free dim

            cid_sb = rpool.tile([128, 1], F32)
            nc.sync.dma_start(cid_sb[:], cid[:])

            # token-major planes [128, NF]: t = f*128 + p
            rtt = rt_all[:].rearrange("(f p) v -> p f v", p=128)
            i1t = rpool.tile([128, NF], F32)
            i2t = rpool.tile([128, NF], F32)
            g1t = rpool.tile([128, NF], F32)
            g2t = rpool.tile([128, NF], F32)
            nc.sync.dma_start(g1t[:], rtt[:, :, 0:1])
            nc.sync.dma_start(g2t[:], rtt[:, :, 1:2])
            nc.sync.dma_start(i1t[:], rtt[:, :, 2:3])
            nc.sync.dma_start(i2t[:], rtt[:, :, 3:4])
            eq1t = rpool.tile([128, NF], F32)
            eq2t = rpool.tile([128, NF], F32)
            nc.vector.tensor_scalar(eq1t[:], i1t[:], cid_sb[:], None, ALU.is_equal)
            nc.vector.tensor_scalar(eq2t[:], i2t[:], cid_sb[:], None, ALU.is_equal)
            m128 = rpool.tile([128, NF], F32)
            nc.vector.tensor_tensor(m128[:], eq1t[:], eq2t[:], ALU.add)
            # combine weight for this expert per token
            cw128 = rpool.tile([128, NF], F32)
            tmpc = rpool.tile([128, NF], F32)
            nc.vector.tensor_tensor(cw128[:], eq1t[:], g1t[:], ALU.mult)
            nc.vector.tensor_tensor(tmpc[:], eq2t[:], g2t[:], ALU.mult)
            nc.vector.tensor_tensor(cw128[:], cw128[:], tmpc[:], ALU.add)

            # pos[t] = # routed tokens before t  (exclusive running count)
            ppref = psT.tile([128, NF], F32, tag="psT")
            nc.tensor.matmul(ppref[:], utri[:], m128[:], start=True, stop=True)
            ptot = psT.tile([128, NF], F32, tag="psT")
            nc.tensor.matmul(ptot[:], ones[:], m128[:], start=True, stop=True)
            zrow = rpool.tile([128, NF], F32)
            nc.vector.memset(zrow[:], 0.0)
            tinc = rpool.tile([128, NF], F32)
            nc.vector.tensor_tensor_scan(
                tinc[:], zrow[:], ptot[:], 0.0, ALU.add, ALU.add
            )
            pos = rpool.tile([128, NF], F32)
            # pos = ppref + (tinc - ptot)  (exclusive block offsets)
            nc.vector.tensor_tensor(pos[:], tinc[:], ptot[:], ALU.subtract)
            nc.vector.tensor_tensor(pos[:], pos[:], ppref[:], ALU.add)
            nc.vector.tensor_scalar_min(pos[:], pos[:], float(C - 1))
            posc = rpool.tile([128, NF], I16)
            nc.vector.tensor_copy(posc[:], pos[:])
            # rewrap [128, NF] (t=f*128+p) -> [16, T/16] (t=f16*16+r)
            posw16 = rpool.tile([16, F16], I16)
            pw = posw16[:].rearrange("p (f a) -> p a f", a=8)
            for a in range(8):
                nc.sync.dma_start(pw[:, a, :], posc[ts(a, 16), :])
            posw = rpool.tile([128, F16], I16)
            for a in range(8):
                nc.sync.dma_start(posw[ts(a, 16), :], posw16[:])

            # dispatch token list: compact routed token ids (wrap layout)
            i1w = rpool.tile([16, F16], F32)
            i2w = rpool.tile([16, F16], F32)
            rtw = rt_all[:].rearrange("(f p) v -> p f v", p=16)
            nc.sync.dma_start(i1w[:], rtw[:, :, 2:3])
            nc.sync.dma_start(i2w[:], rtw[:, :, 3:4])
            eq1 = rpool.tile([16, F16], F32)
            eq2 = rpool.tile([16, F16], F32)
            nc.vector.tensor_scalar(eq1[:], i1w[:], cid_sb[0:16, :], None, ALU.is_equal)
            nc.vector.tensor_scalar(eq2[:], i2w[:], cid_sb[0:16, :], None, ALU.is_equal)
            msk = rpool.tile([16, F16], F32)
            nc.vector.tensor_tensor(msk[:], eq1[:], eq2[:], ALU.add)
            # arr_t = msk*(t+1) - 1  (== t if routed else -1)
            iot = rpool.tile([16, F16], F32)
            nc.sync.dma_start(iot[:], iotw[:])
            arr_t = rpool.tile([16, F16], F32)
            nc.vector.tensor_tensor(arr_t[:], msk[:], iot[:], ALU.mult)
            nc.vector.tensor_scalar_sub(arr_t[:], arr_t[:], 1.0)

            # Q7 ucode libraries: sparse_gather lives in lib "sparse_gather",
            # dma_gather in lib "mlp"; switch inside one critical section.
            idxf = rpool.tile([16, CF], F32)
            nfound1 = rpool.tile([1, 1], U32)
            with tc.tile_critical():
                nc.gpsimd.load_library(library_config.sparse_gather)
                nc.gpsimd.sparse_gather(idxf[:], arr_t[:], num_found=nfound1[:])
                nc.gpsimd.load_library(library_config.mlp)

            # tail -1 -> T (zero pad row of xf)
            mneg = rpool.tile([16, CF], F32)
            nc.vector.tensor_scalar(mneg[:], idxf[:], 0.0, None, ALU.is_lt)
            nc.vector.scalar_tensor_tensor(
                idxf[:], mneg[:], float(T + 1), idxf[:], ALU.mult, ALU.add
            )
            idx16 = rpool.tile([16, CF], I16)
            nc.vector.tensor_copy(idx16[:], idxf[:])
            idx128 = rpool.tile([128, CF], I16)
            for a in range(8):
                nc.sync.dma_start(idx128[ts(a, 16), :], idx16[:])

            # ---------------- expert FFN over capacity slots ----------------
            skip = p.get("skip", ())
            for i in range(NTT):
                xg = xgp.tile([128, NTB, D], F32, tag="xg")
                if "gather" in skip:
                    nc.vector.memset(xg[:], 0.01)
                else:
                    nc.gpsimd.dma_gather(
                        xg[:], xf[:], idx128[:, ds(i * TT // 16, TT // 16)],
                        num_idxs=TT, num_idxs_reg=TT, elem_size=D,
                    )
                xgT = xgtp.tile([128, ND, TT], BF16, tag="xgT")
                if "transpose" in skip:
                    nc.vector.memset(xgT[:], 0.01)
                else:
                    for j in range(NTB):
                        for d in range(ND):
                            ptx = psT.tile([128, 128], F32, tag="psT")
                            nc.tensor.transpose(
                                ptx[:], xg[:, j, ts(d, 128)], ident[:]
                            )
                            nc.scalar.copy(xgT[:, d, ds(j * 128, 128)], ptx[:])
                ysb = yp.tile([128, NTB, D], F32, tag="y")
                if "fc" in skip:
                    nc.vector.tensor_copy(ysb[:], xg[:])
                else:
                    py = psY.tile([128, NTB, D], F32, tag="psY")
                    for h in range(NH):
                        ph = psA.tile([128, TT], F32, tag="psA")
                        for d in range(ND):
                            nc.tensor.matmul(
                                ph[:], w1b[:, d, ts(h, 128)], xgT[:, d, :],
                                start=(d == 0), stop=(d == ND - 1),
                            )
                        ht = htp.tile([128, TT], BF16, tag="ht")
                        actf = AF.Gelu if p.get("act", "gelu") == "gelu" else AF.Identity
                        nc.scalar.activation(ht[:], ph[:], actf)
                        for j in range(NTB):
                            for dt in range(NDT):
                                nc.tensor.matmul(
                                    py[:, j, ds(dt * DT, DT)],
                                    ht[:, ts(j, 128)],
                                    w2b[:, h, ds(dt * DT, DT)],
                                    start=(h == 0), stop=(h == NH - 1),
                                )
                    for j in range(NTB):
                        nc.vector.tensor_copy(ysb[:, j, :], py[:, j, :])
                # dense slot-order write of this tile's expert outputs
                nc.sync.dma_start(
                    ydram[ds(i * TT, TT), :].rearrange("(j q) d -> q j d", q=128),
                    ysb[:],
                )

            # ---------------- combine: gather own slots per token ----------------
            for i in range(T // TT):
                yg = xgp.tile([128, NTB, D], F32, tag="xg")
                nc.gpsimd.dma_gather(
                    yg[:], ydram[:], posw[:, ds(i * TT // 16, TT // 16)],
                    num_idxs=TT, num_idxs_reg=TT, elem_size=D,
                )
                psb = yp.tile([128, NTB, D], F32, tag="y")
                for j in range(NTB):
                    nc.vector.tensor_scalar(
                        psb[:, j, :], yg[:, j, :],
                        cw128[:, i * NTB + j: i * NTB + j + 1], None, ALU.mult,
                    )
                nc.sync.dma_start(
                    partial[ds(i * TT, TT), :].rearrange("(j q) d -> q j d", q=128),
                    psb[:],
                )

            # ---------------- combine across cores ----------------
            nc.gpsimd.collective_compute(
                "ReduceScatter", ALU.add, replica_groups=groups,
                ins=[partial[0:T, :].opt()], outs=[rs_out[:].opt()],
            )
            nc.sync.dma_start(out[:], rs_out[:])

    nc.compile()
    return nc


def make_in_maps(p, x, router_w, w1, w2):
    T, D, NCORES, TC = p["T"], p["D"], p["NCORES"], p["T"] // p["NCORES"]
    xflat = np.ascontiguousarray(x.reshape(T, D), dtype=np.float32)
    xf = np.concatenate([xflat, np.zeros((1, D), np.float32)], axis=0)
    rwt = np.ascontiguousarray(router_w.T, dtype=np.float32)
    iotw = np.ascontiguousarray(
        (np.arange(T, dtype=np.float32) + 1.0).reshape(T // 16, 16).T
    )
    in_maps = []
    for c in range(NCORES):
        in_maps.append(
            {
                "xf": xf,
                "xs": np.ascontiguousarray(xflat[c * TC : (c + 1) * TC]),
                "rwt": rwt,
                "w1": np.ascontiguousarray(w1[c], dtype=np.float32),
                "w2": np.ascontiguousarray(w2[c], dtype=np.float32),
                "cid": np.full((128, 1), c, np.float32),
                "iotw": iotw,
            }
        )
    return in_maps


_CACHE = {}


def _get_nc(key="real"):
    if key not in _CACHE:
        _CACHE[key] = build_moe(REAL)
    return _CACHE[key]


def kernel(x, router_w, w1, w2):
    from concourse import bass_utils

    p = REAL
    nc = _get_nc()
    in_maps = make_in_maps(p, np.asarray(x), np.asarray(router_w),
                           np.asarray(w1), np.asarray(w2))
    res = bass_utils.run_bass_kernel_spmd(
        nc, in_maps, core_ids=list(range(p["NCORES"]))
    )
    outs = [res.results[c]["out"] for c in range(p["NCORES"])]
    full = np.concatenate(outs, axis=0)
    return full.reshape(x.shape).astype(np.float32)


if __name__ == "__main__":
    print("building REAL kernel...")
    build_moe(REAL)
    print("ok")

